# revision 28
# baseline (speedup 1.0000x reference)
"""Tensor-parallel attention kernel for Trainium2 (8 NeuronCores).

Problem: S=2048, B=2, Dm=2048, H=16, Dh=128 attention layer with per-head
RMSNorm (q,k) + RoPE + SDPA + output projection.

Sharding: tensor-parallel over heads. Core c owns heads {2c, 2c+1}:
Wq/Wk/Wv sharded by output rows (256 rows per core), Wo by columns; each
core computes a full-shape partial of the output projection and the host
sums the 8 partials.

Schedule (single fused Tile program, region-interleaved so the Tensor
engine never starves on the Scalar engine's exp throughput):
  R1: QKV+rope for batch 0 (PE-dense, ACT idle)
  R2: QKV+rope for batch 1 interleaved with batch-0 SDPA blocks
  R3: batch-1 SDPA blocks + ALL output projections

Key scheduling properties:
- cos/sin (with the RMSNorm gains folded in) are precomputed on the host
  and uploaded, so the device program starts its weight/x DMAs at t=0 and
  the ACT engine only ever needs one table set (exp; square is a filler).
- q/k transposes run in bf16 (1 PE cycle/row vs 4 for fp32 mode) and are
  software-pipelined one chunk behind the projection matmuls, interleaved
  as [qk(i) trQ(i-1) v(i) trK(i-1)] so the PE never waits on the DVE rope
  chain and a single PSUM bank suffices for the transpose targets.
- softmax denominators accumulate on the DVE (bf16 tree) with a single
  ones-matmul per 512-query block instead of one per key-group.
- all PSUM->SBUF drains are explicitly spread across ACT/DVE.
"""
import sys

for _p in ("/opt/trn_rl_repo", "/root/.axon_site/_ro/trn_rl_repo"):
    if _p not in sys.path:
        sys.path.append(_p)

import math
import numpy as np
import ml_dtypes

import concourse.bass as bass
import concourse.tile as tile
from concourse import bacc, mybir
from concourse import bass_utils

F32 = mybir.dt.float32
BF16 = mybir.dt.bfloat16
I32 = mybir.dt.int32
AF = mybir.ActivationFunctionType
MUL = mybir.AluOpType.mult
ADD = mybir.AluOpType.add
SUB = mybir.AluOpType.subtract

S, B, DM, H, DH = 2048, 2, 2048, 16, 128
NC = 8                 # cores
HC = H // NC           # heads per core = 2
JC = HC * DH           # per-core inner dim = 256
T = S * B              # tokens = 4096
KO = DM // 128         # contraction chunks = 16
TCH = T // 128         # token chunks = 32
SCH = S // 128         # 16 token/key chunks per batch

_CACHE = {}


def _build():
    nc = bacc.Bacc(trn_type="TRN2", target_bir_lowering=False, debug=False,
                   num_devices=NC)

    xT_d = nc.dram_tensor("xT", [TCH, 128, KO, 128], BF16,
                          kind="ExternalInput").ap()
    wqkv_d = nc.dram_tensor("wqkv", [DM, 3 * JC], BF16, kind="ExternalInput").ap()
    wo_d = nc.dram_tensor("woT", [JC, DM], BF16, kind="ExternalInput").ap()
    # host-precomputed cos/sin with rms gains folded: [p, sc, cs, t, f, d]
    cs_d = nc.dram_tensor("cs", [128, SCH, 2, 2, 2, 64], BF16,
                          kind="ExternalInput").ap()
    out_d = nc.dram_tensor("out", [T, DM], BF16, kind="ExternalOutput").ap()

    with tile.TileContext(nc) as tc:
        with tc.tile_pool(name="persist", bufs=1) as persist, \
             tc.tile_pool(name="phb", bufs=1) as phb, \
             tc.tile_pool(name="et", bufs=3) as etp, \
             tc.tile_pool(name="esp", bufs=2) as esp, \
             tc.tile_pool(name="otp", bufs=6) as otp, \
             tc.tile_pool(name="wkb", bufs=2) as wkb, \
             tc.tile_pool(name="ob", bufs=2) as obp, \
             tc.tile_pool(name="ppav", bufs=2, space="PSUM") as ppav:

            # live across regions; bf16: dh on partitions for q,k
            qT = persist.tile([128, HC, T], BF16)
            kT = persist.tile([128, HC, T], BF16)
            v_sb = persist.tile([128, TCH, JC], BF16)  # tokens on partitions

            wo = phb.tile([128, HC, DM], BF16)
            ones = phb.tile([128, 1], BF16)
            nc.vector.memset(ones[:], 1.0)

            outT_of = {}
            pending_fin = []

            def flush_fin(keep=0):
                """Finalize deferred blocks: den matmul + softmax divide.

                Deferred so the den matmul (which waits on the DVE exp-sum
                chain) never blocks the PE queue right at a block boundary."""
                while len(pending_fin) > keep:
                    outT, h, ps_av, acc, den_alloc = pending_fin.pop(0)
                    ps_den = den_alloc()[0:1, :]
                    nc.tensor.matmul(ps_den, ones[:], acc[:],
                                     start=True, stop=True)
                    rec = wkb.tile([1, 512], F32, tag="rec")
                    nc.vector.reciprocal_approx_fast(rec[:], ps_den)
                    recb = wkb.tile([128, 512], F32, tag="recb")
                    nc.gpsimd.partition_broadcast(recb[:], rec[:])
                    nc.vector.tensor_tensor(outT[:, h, :], ps_av[:],
                                            recb[:], MUL)

            def emit_block(b, sj, h, scpool, sctag, nexp, den_alloc):
                """SDPA for one (batch, 512-query block, head)."""
                s0 = b * S + sj * 512
                if h == 0:
                    outT_of[(b, sj)] = otp.tile([128, HC, 512], BF16,
                                                tag="outT", name="outT")
                outT = outT_of[(b, sj)]
                ps_av = ppav.tile([128, 512], F32, tag="psav")
                acc = esp.tile([128, 512], BF16, tag="acc")
                for grp in range(4):  # 4 key-chunks per group
                    eT = etp.tile([128, 4, 512], BF16, tag="eT")
                    for ci in range(4 // nexp):
                        ps_sc = scpool.tile([128, nexp, 512], F32, tag=sctag)
                        for cc in range(nexp):
                            ti = grp * 4 + ci * nexp + cc
                            nc.tensor.matmul(
                                ps_sc[:, cc, :],
                                kT[:, h, b * S + ti * 128:
                                   b * S + (ti + 1) * 128],
                                qT[:, h, s0:s0 + 512],
                                start=True, stop=True)
                        nc.scalar.activation(
                            eT[:, ci * nexp:(ci + 1) * nexp, :],
                            ps_sc[:], AF.Exp)
                    # denominator accumulation fully on DVE (bf16)
                    es = esp.tile([128, 2, 512], BF16, tag="es")
                    nc.vector.tensor_tensor(
                        es[:, 0, :], eT[:, 0, :], eT[:, 1, :], ADD)
                    nc.vector.tensor_tensor(
                        es[:, 1, :], eT[:, 2, :], eT[:, 3, :], ADD)
                    if grp == 0:
                        nc.vector.tensor_tensor(
                            acc[:], es[:, 0, :], es[:, 1, :], ADD)
                    else:
                        esq = esp.tile([128, 512], BF16, tag="esq")
                        nc.vector.tensor_tensor(
                            esq[:], es[:, 0, :], es[:, 1, :], ADD)
                        nc.vector.tensor_tensor(acc[:], acc[:], esq[:], ADD)
                    for cc in range(4):
                        ti = grp * 4 + cc
                        nc.tensor.matmul(
                            ps_av[:],
                            v_sb[:, b * SCH + ti, h * DH:(h + 1) * DH],
                            eT[:, cc, :],
                            start=(ti == 0), stop=(ti == SCH - 1))
                pending_fin.append((outT, h, ps_av, acc, den_alloc))

            def emit_outproj(b, sj, ppo):
                flush_fin(keep=1)
                outT = outT_of[(b, sj)]
                for mi in range(4):  # 128-token rows of the output
                    m0 = b * S + sj * 512 + mi * 128
                    osb = obp.tile([128, DM], BF16, tag="osb")
                    for oj in range(4):
                        ps_o = ppo.tile([128, 512], F32, tag="pso")
                        for h in range(HC):
                            nc.tensor.matmul(
                                ps_o[:],
                                outT[:, h, mi * 128:(mi + 1) * 128],
                                wo[:, h, oj * 512:(oj + 1) * 512],
                                start=(h == 0), stop=(h == HC - 1))
                        dst = osb[:, oj * 512:(oj + 1) * 512]
                        if oj % 2 == 1:
                            nc.scalar.copy(dst, ps_o[:])
                        else:
                            nc.vector.tensor_copy(dst, ps_o[:])
                    oeng = nc.sync if mi % 2 == 0 else nc.gpsimd
                    oeng.dma_start(out_d[m0:m0 + 128, :], osb[:])

            # ---------------- Phase A pools (R1+R2), then R3 pools --------
            with tc.tile_pool(name="pha", bufs=1) as pha, \
                 tc.tile_pool(name="wka", bufs=2) as wka, \
                 tc.tile_pool(name="xin", bufs=3) as xin, \
                 tc.tile_pool(name="ppqk", bufs=2, space="PSUM") as ppqk, \
                 tc.tile_pool(name="ppv", bufs=1, space="PSUM") as ppv, \
                 tc.tile_pool(name="pptr", bufs=1, space="PSUM") as pptr, \
                 tc.tile_pool(name="ppsca", bufs=2, space="PSUM") as ppsca:

                wqkv_src = wqkv_d.rearrange("(ko ki) n -> ki ko n", ki=128)
                wqkv = [pha.tile([128, 3 * JC], BF16, tag=f"wqkv{ko}",
                                 name=f"wqkv{ko}")
                        for ko in range(KO)]
                cs_sb = pha.tile([128, SCH, 2, 2, 2, 64], BF16)
                ident = pha.tile([128, 128], BF16)

                # Startup DMA ordering. HBM bandwidth is fair-shared across
                # all in-flight transfers, so completion time ~ bytes x
                # (number in flight): keep the startup-critical set small.
                # wqkv is split into its qk half (needed by the first
                # matmuls, ko-ordered) and v half (needed half a chunk
                # later); wo (R3-only) is deferred into R1.
                xc0 = xin.tile([128, KO, 128], BF16, tag="xc", name="xc0")
                KQ = KO // 4
                nc.sync.dma_start(xc0[:, 0:KQ, :], xT_d[0, :, 0:KQ, :])
                nc.scalar.dma_start(wqkv[1][:, 0:2 * JC],
                                    wqkv_src[:, 1, 0:2 * JC])
                nc.sync.dma_start(wqkv[0][:, 0:2 * JC],
                                  wqkv_src[:, 0, 0:2 * JC])
                nc.scalar.dma_start(xc0[:, KQ:2 * KQ, :],
                                    xT_d[0, :, KQ:2 * KQ, :])
                nc.gpsimd.dma_start(wqkv[0][:, 2 * JC:],
                                    wqkv_src[:, 0, 2 * JC:])
                nc.gpsimd.dma_start(wqkv[1][:, 2 * JC:],
                                    wqkv_src[:, 1, 2 * JC:])
                for ko in range(2, KO):
                    eng = nc.sync if ko % 2 == 0 else nc.scalar
                    eng.dma_start(wqkv[ko][:, 0:2 * JC],
                                  wqkv_src[:, ko, 0:2 * JC])
                    veng = nc.gpsimd if ko % 2 == 0 else eng
                    veng.dma_start(wqkv[ko][:, 2 * JC:],
                                   wqkv_src[:, ko, 2 * JC:])
                    if ko == 3:
                        nc.sync.dma_start(xc0[:, 2 * KQ:3 * KQ, :],
                                          xT_d[0, :, 2 * KQ:3 * KQ, :])
                        nc.scalar.dma_start(xc0[:, 3 * KQ:, :],
                                            xT_d[0, :, 3 * KQ:, :])
                    if ko == 5:
                        nc.gpsimd.dma_start(cs_sb[:], cs_d[:])

                identf = wka.tile([128, 128], F32, tag="identf")
                from concourse.masks import make_identity
                make_identity(nc, identf[:])
                nc.vector.tensor_copy(ident[:], identf[:])

                def cs_ap(sc, which):
                    # [p, t, h(bcast), (f d)]
                    return (cs_sb[:, sc, which, :, :, :]
                            .rearrange("p t f d -> p t (f d)")
                            [:, :, None, :]
                            .broadcast_to((128, 2, HC, 2 * 64)))

                def emit_qkv(tcch, xc, tr_cb):
                    """x DMA + interleaved q/k and v projection matmuls.

                    tr_cb(t) emits the previous chunk's q (t=0) / k (t=1)
                    transposes; called mid-loop and at the end so the PE
                    pipeline keeps alternating PSUM banks."""
                    if xc is None:
                        xc = xin.tile([128, KO, 128], BF16, tag="xc")
                        half_ko = KO // 2
                        nc.sync.dma_start(xc[:, 0:half_ko, :],
                                          xT_d[tcch, :, 0:half_ko, :])
                        nc.gpsimd.dma_start(xc[:, half_ko:, :],
                                            xT_d[tcch, :, half_ko:, :])
                    ps_qk = ppqk.tile([128, 2 * JC], F32, tag="psqk")
                    # full-bank tile: [:, :JC] holds v; the same tag also
                    # serves as the den-matmul target between chunks
                    ps_v = ppv.tile([128, 512], F32, tag="psv")
                    for ko in range(KO):
                        nc.tensor.matmul(ps_qk[:], xc[:, ko, :],
                                         wqkv[ko][:, 0:2 * JC],
                                         start=(ko == 0), stop=(ko == KO - 1))
                        nc.tensor.matmul(ps_v[:, 0:JC], xc[:, ko, :],
                                         wqkv[ko][:, 2 * JC:3 * JC],
                                         start=(ko == 0), stop=(ko == KO - 1))
                        if ko == KO // 2 - 1:
                            tr_cb(0)
                    tr_cb(1)
                    nc.scalar.copy(v_sb[:, tcch, :], ps_v[:, 0:JC])
                    return ps_qk

                def emit_chain(tcch, ps_qk):
                    """ACT/DVE chain: rmsnorm stats + rope -> trr (bf16)."""
                    sc = tcch % SCH
                    qk_sb = wka.tile([128, 2, HC, 2, 64], BF16, tag="qksb")
                    nc.scalar.copy(
                        qk_sb[:].rearrange("p t h f d -> p (t h f d)"),
                        ps_qk[:])
                    sq = wka.tile([128, 2 * JC], F32, tag="sq")
                    nc.scalar.square(sq[:], ps_qk[:])
                    ssq = wka.tile([128, 4], F32, tag="ssq")
                    nc.vector.tensor_reduce(
                        ssq[:], sq[:].rearrange("p (g d) -> p g d", d=DH),
                        mybir.AxisListType.X, ADD)
                    # rsqrt via bit-trick + one Newton step (no ACT table);
                    # Newton fused into scalar_tensor_tensor ops:
                    #   a = -0.5*ssq*y0 ; d = a*y0 ; rr2 = (d+1.5)*y0
                    # (sqrt(DH) attention scale is folded into the host cs
                    # arrays on the k side, so rr2 is a plain rsqrt here.)
                    y0 = wka.tile([128, 4], I32, tag="y0")
                    nc.vector.tensor_scalar(
                        y0[:], ssq[:].bitcast(I32), 1, -1,
                        mybir.AluOpType.logical_shift_right,
                        mybir.AluOpType.bitwise_xor)
                    nc.vector.tensor_scalar(y0[:], y0[:], 0x5f3759e0, None,
                                            ADD)
                    y0f = y0[:].bitcast(F32)
                    aa = wka.tile([128, 4], F32, tag="aa")
                    nc.vector.scalar_tensor_tensor(aa[:], ssq[:], -0.5, y0f,
                                                   MUL, MUL)
                    dd = wka.tile([128, 4], F32, tag="dd")
                    nc.vector.tensor_tensor(dd[:], aa[:], y0f, MUL)
                    rr2 = wka.tile([128, 4], F32, tag="rr2")
                    nc.vector.scalar_tensor_tensor(rr2[:], dd[:], 1.5, y0f,
                                                   ADD, MUL)
                    # on DVE (not ACT): keeps the in-order ACT queue free of
                    # DVE dependencies so exps never convoy behind it
                    rr2b = wka.tile([128, 4], BF16, tag="rr2b")
                    nc.vector.tensor_copy(rr2b[:], rr2[:])

                    vw = lambda ap: ap.rearrange("p t h f d -> p t h (f d)")
                    tmc = wka.tile([128, 2, HC, 2, 64], BF16, tag="tmc")
                    tms = wka.tile([128, 2, HC, 2, 64], BF16, tag="tms")
                    nc.vector.tensor_tensor(vw(tmc[:]), vw(qk_sb[:]),
                                            cs_ap(sc, 0), MUL)
                    nc.vector.tensor_tensor(vw(tms[:]), vw(qk_sb[:]),
                                            cs_ap(sc, 1), MUL)
                    tr = wka.tile([128, 2, HC, 2, 64], BF16, tag="tr")
                    nc.vector.tensor_tensor(tr[:, :, :, 0, :],
                                            tmc[:, :, :, 0, :],
                                            tms[:, :, :, 1, :], SUB)
                    nc.vector.tensor_tensor(tr[:, :, :, 1, :],
                                            tms[:, :, :, 0, :],
                                            tmc[:, :, :, 1, :], ADD)
                    trr = wka.tile([128, 2 * HC, DH], BF16, tag="trr")
                    nc.vector.tensor_tensor(
                        trr[:], tr[:].rearrange("p t h f d -> p (t h) (f d)"),
                        rr2b[:, :, None].broadcast_to((128, 2 * HC, DH)),
                        MUL)
                    return trr

                def emit_tr(tcch, trr, t):
                    """PE transpose of q (t=0) or k (t=1) pair -> qT/kT."""
                    dstT = qT if t == 0 else kT
                    ps_tr = pptr.tile([128, HC, 128], BF16, tag="pstr",
                                      padded_shape=[128, HC, 512])
                    for hh in range(HC):
                        nc.tensor.matmul(ps_tr[:, hh, :], trr[:, t * HC + hh, :],
                                         ident[:], is_transpose=True,
                                         start=(hh == 0), stop=(hh == HC - 1))
                    nc.vector.tensor_copy(
                        dstT[:, :, tcch * 128:(tcch + 1) * 128], ps_tr[:])

                # software pipeline: chunk i's matmuls interleave with chunk
                # i-1's transposes so the PE never waits on the DVE chain.
                prev = None  # (tcch, trr)

                def emit_tcch(tcch, xc=None):
                    nonlocal prev
                    pv = prev

                    def tr_cb(t):
                        if pv is not None:
                            emit_tr(pv[0], pv[1], t)

                    ps_qk = emit_qkv(tcch, xc, tr_cb)
                    trr = emit_chain(tcch, ps_qk)
                    prev = (tcch, trr)

                # R1: batch 0 projections
                emit_tcch(0, xc0)
                for tcch in range(1, SCH):
                    emit_tcch(tcch)
                    if tcch == 2:
                        # wo is only needed in R3; load it once the startup
                        # DMA crunch is over
                        wo_src = wo_d.rearrange("(h ki) n -> ki h n", ki=128)
                        for h in range(HC):
                            nc.gpsimd.dma_start(wo[:, h, :], wo_src[:, h, :])
                # R2: batch 1 projections interleaved w/ batch-0 SDPA
                den_a = lambda: ppv.tile([128, 512], F32, tag="psv",
                                         name="dent")
                for i in range(8):
                    emit_tcch(SCH + 2 * i)
                    emit_tcch(SCH + 2 * i + 1)
                    flush_fin()
                    emit_block(0, i // 2, i % 2, ppsca, "pssca", 1, den_a)
                # drain the transpose pipeline and any pending block
                # finalize (its psum borrow comes from a phase-A pool)
                emit_tr(prev[0], prev[1], 0)
                emit_tr(prev[0], prev[1], 1)
                flush_fin()

            # R3: batch-1 SDPA + all output projections
            with tc.tile_pool(name="ppscb", bufs=2, space="PSUM") as ppscb, \
                 tc.tile_pool(name="ppo", bufs=4, space="PSUM") as ppo:
                den_b = lambda: ppo.tile([128, 512], F32, tag="pso",
                                         name="dent")
                for sj in range(3):
                    emit_block(1, sj, 0, ppscb, "psscb", 1, den_b)
                    flush_fin(keep=1)
                    emit_block(1, sj, 1, ppscb, "psscb", 1, den_b)
                    flush_fin(keep=1)
                emit_outproj(0, 0, ppo)
                emit_outproj(0, 1, ppo)
                emit_block(1, 3, 0, ppscb, "psscb", 1, den_b)
                flush_fin(keep=1)
                emit_block(1, 3, 1, ppscb, "psscb", 1, den_b)
                emit_outproj(0, 2, ppo)
                emit_outproj(0, 3, ppo)
                flush_fin()
                for sj in range(4):
                    emit_outproj(1, sj, ppo)

    nc.compile()
    return nc


def _get_program():
    if "prog" not in _CACHE:
        _CACHE["prog"] = _build()
    return _CACHE["prog"]


def _prep_inputs(x, rope_emb, Wq, Wk, Wv, Wo, gq, gk):
    x = np.asarray(x, dtype=np.float32)
    # b-major tokens: row r = b*S + s
    xbm = x.transpose(1, 0, 2).reshape(T, DM)
    xT = np.ascontiguousarray(
        xbm.reshape(TCH, 128, KO, 128).transpose(0, 3, 2, 1)
        .astype(ml_dtypes.bfloat16))
    rope = np.asarray(rope_emb, dtype=np.float32).reshape(S, DH)[:, :DH // 2]
    cosv = np.cos(rope).reshape(SCH, 128, 64).transpose(1, 0, 2)  # [p,sc,d]
    sinv = np.sin(rope).reshape(SCH, 128, 64).transpose(1, 0, 2)
    gq1 = np.asarray(gq, dtype=np.float32).reshape(2, 64)  # [f, d]
    gk1 = np.asarray(gk, dtype=np.float32).reshape(2, 64)
    cs = np.empty((128, SCH, 2, 2, 2, 64), np.float32)
    for t, g in enumerate((gq1, gk1)):
        tsc = 1.0 if t == 0 else float(np.sqrt(DH))  # attn scale on k side
        for f in range(2):
            cs[:, :, 0, t, f, :] = cosv * (g[f] * tsc)
            cs[:, :, 1, t, f, :] = sinv * (g[f] * tsc)
    cs = np.ascontiguousarray(cs.astype(ml_dtypes.bfloat16))
    Wq = np.asarray(Wq, dtype=np.float32)
    Wk = np.asarray(Wk, dtype=np.float32)
    Wv = np.asarray(Wv, dtype=np.float32)
    Wo = np.asarray(Wo, dtype=np.float32)
    in_maps = []
    for c in range(NC):
        r0, r1 = c * JC, (c + 1) * JC
        wqkv = np.ascontiguousarray(
            np.concatenate([Wq[r0:r1].T, Wk[r0:r1].T, Wv[r0:r1].T], axis=1)
            .astype(ml_dtypes.bfloat16))
        woT = np.ascontiguousarray(
            Wo[:, r0:r1].T.astype(ml_dtypes.bfloat16))
        in_maps.append({"xT": xT, "wqkv": wqkv, "woT": woT, "cs": cs})
    return in_maps


def _gather(results):
    acc = results[0]["out"].astype(np.float64)
    for r in results[1:]:
        acc += r["out"].astype(np.float64)
    out = acc.astype(np.float32).reshape(B, S, DM).transpose(1, 0, 2)
    return np.ascontiguousarray(out)


def kernel(x, rope_emb, Wq, Wk, Wv, Wo, gq, gk):
    in_maps = _prep_inputs(x, rope_emb, Wq, Wk, Wv, Wo, gq, gk)
    nc = _get_program()
    res = bass_utils.run_bass_kernel_spmd(nc, in_maps, core_ids=list(range(NC)))
    return _gather(res.results)


def kernel_profiled(x, rope_emb, Wq, Wk, Wv, Wo, gq, gk):
    """Like kernel() but with NTFF tracing; returns (out, exec_time_ns)."""
    _install_ntff()
    in_maps = _prep_inputs(x, rope_emb, Wq, Wk, Wv, Wo, gq, gk)
    nc = _get_program()
    res = bass_utils.run_bass_kernel_spmd(nc, in_maps, core_ids=list(range(NC)),
                                          trace=True)
    return _gather(res.results), res.exec_time_ns


def _install_ntff():
    import contextlib
    import ctypes
    import types

    if "antenv.axon_hooks" in sys.modules:
        return
    so_path = "/opt/axon/libaxon_pjrt.so"
    try:
        lib = ctypes.CDLL(so_path)
    except OSError:
        return
    if not hasattr(lib, "axon_start_nrt_profile"):
        return
    lib.axon_start_nrt_profile.argtypes = [ctypes.POINTER(ctypes.c_int64),
                                           ctypes.c_size_t]
    lib.axon_start_nrt_profile.restype = ctypes.c_int64
    lib.axon_stop_nrt_profile.argtypes = [ctypes.c_char_p]
    lib.axon_stop_nrt_profile.restype = ctypes.c_int64

    @contextlib.contextmanager
    def hook(output_dir, device_ids):
        import jax
        jax.devices()
        if device_ids:
            ids = (ctypes.c_int64 * len(device_ids))(*device_ids)
            rc = lib.axon_start_nrt_profile(ids, len(device_ids))
        else:
            rc = lib.axon_start_nrt_profile(None, 0)
        if rc != 0:
            raise RuntimeError(f"axon_start_nrt_profile rc={rc}")
        try:
            yield
        finally:
            n = lib.axon_stop_nrt_profile(str(output_dir).encode())
            print(f"ntff profile: {n} file(s) -> {output_dir}", file=sys.stderr)

    mod = types.ModuleType("antenv.axon_hooks")
    _state = {"h": hook}
    mod.get_axon_ntff_profile_hook = lambda: _state["h"]
    mod.set_axon_ntff_profile_hook = lambda h: _state.__setitem__("h", h)
    sys.modules["antenv.axon_hooks"] = mod


# revision 29
# speedup vs baseline: 1.0418x; 1.0418x over previous
"""Tensor-parallel attention kernel for Trainium2 (8 NeuronCores).

Problem: S=2048, B=2, Dm=2048, H=16, Dh=128 attention layer with per-head
RMSNorm (q,k) + RoPE + SDPA + output projection.

Sharding: tensor-parallel over heads. Core c owns heads {2c, 2c+1}:
Wq/Wk/Wv sharded by output rows (256 rows per core), Wo by columns; each
core computes a full-shape partial of the output projection and the host
sums the 8 partials.

Schedule (single fused Tile program, region-interleaved so the Tensor
engine never starves on the Scalar engine's exp throughput):
  R1: QKV+rope for batch 0 (PE-dense, ACT idle)
  R2: QKV+rope for batch 1 interleaved with batch-0 SDPA blocks
  R3: batch-1 SDPA blocks + ALL output projections

Key scheduling properties:
- cos/sin (with the RMSNorm gains folded in) are precomputed on the host
  and uploaded, so the device program starts its weight/x DMAs at t=0 and
  the ACT engine only ever needs one table set (exp; square is a filler).
- q/k transposes run in bf16 (1 PE cycle/row vs 4 for fp32 mode) and are
  software-pipelined one chunk behind the projection matmuls, interleaved
  as [qk(i) trQ(i-1) v(i) trK(i-1)] so the PE never waits on the DVE rope
  chain and a single PSUM bank suffices for the transpose targets.
- softmax denominators accumulate on the DVE (bf16 tree) with a single
  ones-matmul per 512-query block instead of one per key-group.
- all PSUM->SBUF drains are explicitly spread across ACT/DVE.
"""
import sys

for _p in ("/opt/trn_rl_repo", "/root/.axon_site/_ro/trn_rl_repo"):
    if _p not in sys.path:
        sys.path.append(_p)

import math
import numpy as np
import ml_dtypes

import concourse.bass as bass
import concourse.tile as tile
from concourse import bacc, mybir
from concourse import bass_utils

F32 = mybir.dt.float32
BF16 = mybir.dt.bfloat16
I32 = mybir.dt.int32
AF = mybir.ActivationFunctionType
MUL = mybir.AluOpType.mult
ADD = mybir.AluOpType.add
SUB = mybir.AluOpType.subtract

S, B, DM, H, DH = 2048, 2, 2048, 16, 128
NC = 8                 # cores
HC = H // NC           # heads per core = 2
JC = HC * DH           # per-core inner dim = 256
T = S * B              # tokens = 4096
KO = DM // 128         # contraction chunks = 16
TCH = T // 128         # token chunks = 32
SCH = S // 128         # 16 token/key chunks per batch

_CACHE = {}


def _build():
    nc = bacc.Bacc(trn_type="TRN2", target_bir_lowering=False, debug=False,
                   num_devices=NC)

    xT_d = nc.dram_tensor("xT", [TCH, 128, KO, 128], BF16,
                          kind="ExternalInput").ap()
    wqkv_d = nc.dram_tensor("wqkv", [DM, 3 * JC], BF16, kind="ExternalInput").ap()
    wo_d = nc.dram_tensor("woT", [JC, DM], BF16, kind="ExternalInput").ap()
    # host-precomputed cos/sin with rms gains folded: [p, sc, cs, t, f, d]
    cs_d = nc.dram_tensor("cs", [128, SCH, 2, 2, 2, 64], BF16,
                          kind="ExternalInput").ap()
    out_d = nc.dram_tensor("out", [T, DM], BF16, kind="ExternalOutput").ap()

    with tile.TileContext(nc) as tc:
        with tc.tile_pool(name="persist", bufs=1) as persist, \
             tc.tile_pool(name="phb", bufs=1) as phb, \
             tc.tile_pool(name="et", bufs=3) as etp, \
             tc.tile_pool(name="esp", bufs=2) as esp, \
             tc.tile_pool(name="otp", bufs=6) as otp, \
             tc.tile_pool(name="wkb", bufs=2) as wkb, \
             tc.tile_pool(name="ob", bufs=2) as obp, \
             tc.tile_pool(name="ppav", bufs=2, space="PSUM") as ppav:

            # live across regions; bf16: dh on partitions for q,k
            qT = persist.tile([128, HC, T], BF16)
            kT = persist.tile([128, HC, T], BF16)
            v_sb = persist.tile([128, TCH, JC], BF16)  # tokens on partitions

            wo = phb.tile([128, HC, DM], BF16)
            ones = phb.tile([128, 1], BF16)
            nc.vector.memset(ones[:], 1.0)

            outT_of = {}
            pending_fin = []

            def flush_fin(keep=0):
                """Finalize deferred blocks: den matmul + softmax divide.

                Deferred so the den matmul (which waits on the DVE exp-sum
                chain) never blocks the PE queue right at a block boundary."""
                while len(pending_fin) > keep:
                    outT, h, ps_av, acc, den_alloc = pending_fin.pop(0)
                    ps_den = den_alloc()[0:1, :]
                    nc.tensor.matmul(ps_den, ones[:], acc[:],
                                     start=True, stop=True)
                    rec = wkb.tile([1, 512], F32, tag="rec")
                    nc.vector.reciprocal_approx_fast(rec[:], ps_den)
                    recb = wkb.tile([128, 512], F32, tag="recb")
                    nc.gpsimd.partition_broadcast(recb[:], rec[:])
                    nc.vector.tensor_tensor(outT[:, h, :], ps_av[:],
                                            recb[:], MUL)

            def emit_block(b, sj, h, scpool, sctag, nexp, den_alloc):
                """SDPA for one (batch, 512-query block, head)."""
                s0 = b * S + sj * 512
                if h == 0:
                    outT_of[(b, sj)] = otp.tile([128, HC, 512], BF16,
                                                tag="outT", name="outT")
                outT = outT_of[(b, sj)]
                ps_av = ppav.tile([128, 512], F32, tag="psav")
                acc = esp.tile([128, 512], BF16, tag="acc")
                for grp in range(4):  # 4 key-chunks per group
                    eT = etp.tile([128, 4, 512], BF16, tag="eT")
                    for ci in range(4 // nexp):
                        ps_sc = scpool.tile([128, nexp, 512], F32, tag=sctag)
                        for cc in range(nexp):
                            ti = grp * 4 + ci * nexp + cc
                            nc.tensor.matmul(
                                ps_sc[:, cc, :],
                                kT[:, h, b * S + ti * 128:
                                   b * S + (ti + 1) * 128],
                                qT[:, h, s0:s0 + 512],
                                start=True, stop=True)
                        nc.scalar.activation(
                            eT[:, ci * nexp:(ci + 1) * nexp, :],
                            ps_sc[:], AF.Exp)
                    # denominator accumulation fully on DVE (bf16)
                    es = esp.tile([128, 2, 512], BF16, tag="es")
                    nc.vector.tensor_tensor(
                        es[:, 0, :], eT[:, 0, :], eT[:, 1, :], ADD)
                    nc.vector.tensor_tensor(
                        es[:, 1, :], eT[:, 2, :], eT[:, 3, :], ADD)
                    if grp == 0:
                        nc.vector.tensor_tensor(
                            acc[:], es[:, 0, :], es[:, 1, :], ADD)
                    else:
                        esq = esp.tile([128, 512], BF16, tag="esq")
                        nc.vector.tensor_tensor(
                            esq[:], es[:, 0, :], es[:, 1, :], ADD)
                        nc.vector.tensor_tensor(acc[:], acc[:], esq[:], ADD)
                    for cc in range(4):
                        ti = grp * 4 + cc
                        nc.tensor.matmul(
                            ps_av[:],
                            v_sb[:, b * SCH + ti, h * DH:(h + 1) * DH],
                            eT[:, cc, :],
                            start=(ti == 0), stop=(ti == SCH - 1))
                pending_fin.append((outT, h, ps_av, acc, den_alloc))

            def emit_outproj(b, sj, ppo):
                flush_fin(keep=1)
                outT = outT_of[(b, sj)]
                for mi in range(4):  # 128-token rows of the output
                    m0 = b * S + sj * 512 + mi * 128
                    osb = obp.tile([128, DM], BF16, tag="osb")
                    for oj in range(4):
                        ps_o = ppo.tile([128, 512], F32, tag="pso")
                        for h in range(HC):
                            nc.tensor.matmul(
                                ps_o[:],
                                outT[:, h, mi * 128:(mi + 1) * 128],
                                wo[:, h, oj * 512:(oj + 1) * 512],
                                start=(h == 0), stop=(h == HC - 1))
                        dst = osb[:, oj * 512:(oj + 1) * 512]
                        if oj % 2 == 1:
                            nc.scalar.copy(dst, ps_o[:])
                        else:
                            nc.vector.tensor_copy(dst, ps_o[:])
                    oeng = nc.sync if mi % 2 == 0 else nc.gpsimd
                    oeng.dma_start(out_d[m0:m0 + 128, :], osb[:])

            # ---------------- Phase A pools (R1+R2), then R3 pools --------
            with tc.tile_pool(name="pha", bufs=1) as pha, \
                 tc.tile_pool(name="wka", bufs=2) as wka, \
                 tc.tile_pool(name="xin", bufs=3) as xin, \
                 tc.tile_pool(name="ppqk", bufs=2, space="PSUM") as ppqk, \
                 tc.tile_pool(name="ppv", bufs=1, space="PSUM") as ppv, \
                 tc.tile_pool(name="pptr", bufs=1, space="PSUM") as pptr, \
                 tc.tile_pool(name="ppsca", bufs=2, space="PSUM") as ppsca:

                wqkv_src = wqkv_d.rearrange("(ko ki) n -> ki ko n", ki=128)
                wqkv = [pha.tile([128, 3 * JC], BF16, tag=f"wqkv{ko}",
                                 name=f"wqkv{ko}")
                        for ko in range(KO)]
                cs_sb = pha.tile([128, SCH, 2, 2, 2, 64], BF16)
                ident = pha.tile([128, 128], BF16)

                # Startup DMA ordering. HBM bandwidth is fair-shared across
                # all in-flight transfers, so completion time ~ bytes x
                # (number in flight): keep the startup-critical set small.
                # wqkv is split into its qk half (needed by the first
                # matmuls, ko-ordered) and v half (needed half a chunk
                # later); wo (R3-only) is deferred into R1.
                xc0 = xin.tile([128, KO, 128], BF16, tag="xc", name="xc0")
                KQ = KO // 4
                nc.sync.dma_start(xc0[:, 0:KQ, :], xT_d[0, :, 0:KQ, :])
                nc.scalar.dma_start(wqkv[1][:, 0:2 * JC],
                                    wqkv_src[:, 1, 0:2 * JC])
                nc.sync.dma_start(wqkv[0][:, 0:2 * JC],
                                  wqkv_src[:, 0, 0:2 * JC])
                nc.scalar.dma_start(xc0[:, KQ:2 * KQ, :],
                                    xT_d[0, :, KQ:2 * KQ, :])
                nc.gpsimd.dma_start(wqkv[0][:, 2 * JC:],
                                    wqkv_src[:, 0, 2 * JC:])
                nc.gpsimd.dma_start(wqkv[1][:, 2 * JC:],
                                    wqkv_src[:, 1, 2 * JC:])
                for ko in range(2, KO):
                    eng = nc.sync if ko % 2 == 0 else nc.scalar
                    eng.dma_start(wqkv[ko][:, 0:2 * JC],
                                  wqkv_src[:, ko, 0:2 * JC])
                    veng = nc.gpsimd if ko % 2 == 0 else eng
                    veng.dma_start(wqkv[ko][:, 2 * JC:],
                                   wqkv_src[:, ko, 2 * JC:])
                    if ko == 3:
                        nc.sync.dma_start(xc0[:, 2 * KQ:3 * KQ, :],
                                          xT_d[0, :, 2 * KQ:3 * KQ, :])
                        nc.scalar.dma_start(xc0[:, 3 * KQ:, :],
                                            xT_d[0, :, 3 * KQ:, :])
                    if ko == 5:
                        nc.gpsimd.dma_start(cs_sb[:], cs_d[:])

                identf = wka.tile([128, 128], F32, tag="identf")
                from concourse.masks import make_identity
                make_identity(nc, identf[:])
                nc.vector.tensor_copy(ident[:], identf[:])

                def cs_ap(sc, which):
                    # [p, t, h(bcast), (f d)]
                    return (cs_sb[:, sc, which, :, :, :]
                            .rearrange("p t f d -> p t (f d)")
                            [:, :, None, :]
                            .broadcast_to((128, 2, HC, 2 * 64)))

                def emit_qkv(tcch, xc, tr_cb):
                    """x DMA + interleaved q/k and v projection matmuls.

                    tr_cb(t) emits the previous chunk's q (t=0) / k (t=1)
                    transposes; called mid-loop and at the end so the PE
                    pipeline keeps alternating PSUM banks."""
                    if xc is None:
                        xc = xin.tile([128, KO, 128], BF16, tag="xc")
                        half_ko = KO // 2
                        nc.sync.dma_start(xc[:, 0:half_ko, :],
                                          xT_d[tcch, :, 0:half_ko, :])
                        nc.gpsimd.dma_start(xc[:, half_ko:, :],
                                            xT_d[tcch, :, half_ko:, :])
                    ps_qk = ppqk.tile([128, 2 * JC], F32, tag="psqk")
                    # full-bank tile: [:, :JC] holds v; the same tag also
                    # serves as the den-matmul target between chunks
                    ps_v = ppv.tile([128, 512], F32, tag="psv")
                    for ko in range(KO):
                        nc.tensor.matmul(ps_qk[:], xc[:, ko, :],
                                         wqkv[ko][:, 0:2 * JC],
                                         start=(ko == 0), stop=(ko == KO - 1))
                        nc.tensor.matmul(ps_v[:, 0:JC], xc[:, ko, :],
                                         wqkv[ko][:, 2 * JC:3 * JC],
                                         start=(ko == 0), stop=(ko == KO - 1))
                        if ko == KO // 2 - 1:
                            tr_cb(0)
                    tr_cb(1)
                    nc.scalar.copy(v_sb[:, tcch, :], ps_v[:, 0:JC])
                    return ps_qk

                def emit_chain(tcch, ps_qk):
                    """ACT/DVE chain: rmsnorm stats + rope -> trr (bf16)."""
                    sc = tcch % SCH
                    qk_sb = wka.tile([128, 2, HC, 2, 64], BF16, tag="qksb")
                    nc.scalar.copy(
                        qk_sb[:].rearrange("p t h f d -> p (t h f d)"),
                        ps_qk[:])
                    sq = wka.tile([128, 2 * JC], F32, tag="sq")
                    nc.scalar.square(sq[:], ps_qk[:])
                    ssq = wka.tile([128, 4], F32, tag="ssq")
                    nc.vector.tensor_reduce(
                        ssq[:], sq[:].rearrange("p (g d) -> p g d", d=DH),
                        mybir.AxisListType.X, ADD)
                    # rsqrt via bit-trick + one Newton step (no ACT table);
                    # Newton fused into scalar_tensor_tensor ops:
                    #   a = -0.5*ssq*y0 ; d = a*y0 ; rr2 = (d+1.5)*y0
                    # (sqrt(DH) attention scale is folded into the host cs
                    # arrays on the k side, so rr2 is a plain rsqrt here.)
                    y0 = wka.tile([128, 4], I32, tag="y0")
                    nc.vector.tensor_scalar(
                        y0[:], ssq[:].bitcast(I32), 1, -1,
                        mybir.AluOpType.logical_shift_right,
                        mybir.AluOpType.bitwise_xor)
                    nc.vector.tensor_scalar(y0[:], y0[:], 0x5f3759e0, None,
                                            ADD)
                    y0f = y0[:].bitcast(F32)
                    aa = wka.tile([128, 4], F32, tag="aa")
                    nc.vector.scalar_tensor_tensor(aa[:], ssq[:], -0.5, y0f,
                                                   MUL, MUL)
                    dd = wka.tile([128, 4], F32, tag="dd")
                    nc.vector.tensor_tensor(dd[:], aa[:], y0f, MUL)
                    rr2 = wka.tile([128, 4], F32, tag="rr2")
                    nc.vector.scalar_tensor_tensor(rr2[:], dd[:], 1.5, y0f,
                                                   ADD, MUL)
                    # on DVE (not ACT): keeps the in-order ACT queue free of
                    # DVE dependencies so exps never convoy behind it
                    rr2b = wka.tile([128, 4], BF16, tag="rr2b")
                    nc.vector.tensor_copy(rr2b[:], rr2[:])

                    vw = lambda ap: ap.rearrange("p t h f d -> p t h (f d)")
                    tmc = wka.tile([128, 2, HC, 2, 64], BF16, tag="tmc")
                    tms = wka.tile([128, 2, HC, 2, 64], BF16, tag="tms")
                    nc.vector.tensor_tensor(vw(tmc[:]), vw(qk_sb[:]),
                                            cs_ap(sc, 0), MUL)
                    nc.vector.tensor_tensor(vw(tms[:]), vw(qk_sb[:]),
                                            cs_ap(sc, 1), MUL)
                    tr = wka.tile([128, 2, HC, 2, 64], BF16, tag="tr")
                    nc.vector.tensor_tensor(tr[:, :, :, 0, :],
                                            tmc[:, :, :, 0, :],
                                            tms[:, :, :, 1, :], SUB)
                    nc.vector.tensor_tensor(tr[:, :, :, 1, :],
                                            tms[:, :, :, 0, :],
                                            tmc[:, :, :, 1, :], ADD)
                    trr = wka.tile([128, 2 * HC, DH], BF16, tag="trr")
                    nc.vector.tensor_tensor(
                        trr[:], tr[:].rearrange("p t h f d -> p (t h) (f d)"),
                        rr2b[:, :, None].broadcast_to((128, 2 * HC, DH)),
                        MUL)
                    return trr

                def emit_tr(tcch, trr, t):
                    """PE transpose of q (t=0) or k (t=1) pair -> qT/kT."""
                    dstT = qT if t == 0 else kT
                    ps_tr = pptr.tile([128, HC, 128], BF16, tag="pstr",
                                      padded_shape=[128, HC, 512])
                    for hh in range(HC):
                        nc.tensor.matmul(ps_tr[:, hh, :], trr[:, t * HC + hh, :],
                                         ident[:], is_transpose=True,
                                         start=(hh == 0), stop=(hh == HC - 1))
                    nc.vector.tensor_copy(
                        dstT[:, :, tcch * 128:(tcch + 1) * 128], ps_tr[:])

                # software pipeline: chunk i's matmuls interleave with chunk
                # i-1's transposes so the PE never waits on the DVE chain.
                prev = None  # (tcch, trr)

                def emit_tcch(tcch, xc=None):
                    nonlocal prev
                    pv = prev

                    def tr_cb(t):
                        if pv is not None:
                            emit_tr(pv[0], pv[1], t)

                    ps_qk = emit_qkv(tcch, xc, tr_cb)
                    trr = emit_chain(tcch, ps_qk)
                    prev = (tcch, trr)

                # R1: batch 0 projections
                emit_tcch(0, xc0)
                for tcch in range(1, SCH):
                    emit_tcch(tcch)
                    if tcch == 2:
                        # wo is only needed in R3; load it once the startup
                        # DMA crunch is over
                        wo_src = wo_d.rearrange("(h ki) n -> ki h n", ki=128)
                        for h in range(HC):
                            nc.gpsimd.dma_start(wo[:, h, :], wo_src[:, h, :])
                # R2: batch 1 projections interleaved w/ batch-0 SDPA
                den_a = lambda: ppv.tile([128, 512], F32, tag="psv",
                                         name="dent")
                for i in range(8):
                    emit_tcch(SCH + 2 * i)
                    emit_tcch(SCH + 2 * i + 1)
                    flush_fin()
                    emit_block(0, i // 2, i % 2, ppsca, "pssca", 1, den_a)
                # drain the transpose pipeline and any pending block
                # finalize (its psum borrow comes from a phase-A pool)
                emit_tr(prev[0], prev[1], 0)
                emit_tr(prev[0], prev[1], 1)
                flush_fin()

            # R3: batch-1 SDPA + all output projections
            with tc.tile_pool(name="ppscb", bufs=2, space="PSUM") as ppscb, \
                 tc.tile_pool(name="ppo", bufs=2, space="PSUM") as ppo:
                den_b = lambda: ppscb.tile([128, 2, 512], F32, tag="psscb",
                                           name="dent")[:, 0, :]
                for sj in range(3):
                    emit_block(1, sj, 0, ppscb, "psscb", 2, den_b)
                    flush_fin(keep=1)
                    emit_block(1, sj, 1, ppscb, "psscb", 2, den_b)
                    flush_fin(keep=1)
                emit_outproj(0, 0, ppo)
                emit_outproj(0, 1, ppo)
                emit_block(1, 3, 0, ppscb, "psscb", 2, den_b)
                flush_fin(keep=1)
                emit_block(1, 3, 1, ppscb, "psscb", 2, den_b)
                emit_outproj(0, 2, ppo)
                emit_outproj(0, 3, ppo)
                flush_fin()
                for sj in range(4):
                    emit_outproj(1, sj, ppo)

    nc.compile()
    return nc


def _get_program():
    if "prog" not in _CACHE:
        _CACHE["prog"] = _build()
    return _CACHE["prog"]


def _prep_inputs(x, rope_emb, Wq, Wk, Wv, Wo, gq, gk):
    x = np.asarray(x, dtype=np.float32)
    # b-major tokens: row r = b*S + s
    xbm = x.transpose(1, 0, 2).reshape(T, DM)
    xT = np.ascontiguousarray(
        xbm.reshape(TCH, 128, KO, 128).transpose(0, 3, 2, 1)
        .astype(ml_dtypes.bfloat16))
    rope = np.asarray(rope_emb, dtype=np.float32).reshape(S, DH)[:, :DH // 2]
    cosv = np.cos(rope).reshape(SCH, 128, 64).transpose(1, 0, 2)  # [p,sc,d]
    sinv = np.sin(rope).reshape(SCH, 128, 64).transpose(1, 0, 2)
    gq1 = np.asarray(gq, dtype=np.float32).reshape(2, 64)  # [f, d]
    gk1 = np.asarray(gk, dtype=np.float32).reshape(2, 64)
    cs = np.empty((128, SCH, 2, 2, 2, 64), np.float32)
    for t, g in enumerate((gq1, gk1)):
        tsc = 1.0 if t == 0 else float(np.sqrt(DH))  # attn scale on k side
        for f in range(2):
            cs[:, :, 0, t, f, :] = cosv * (g[f] * tsc)
            cs[:, :, 1, t, f, :] = sinv * (g[f] * tsc)
    cs = np.ascontiguousarray(cs.astype(ml_dtypes.bfloat16))
    Wq = np.asarray(Wq, dtype=np.float32)
    Wk = np.asarray(Wk, dtype=np.float32)
    Wv = np.asarray(Wv, dtype=np.float32)
    Wo = np.asarray(Wo, dtype=np.float32)
    in_maps = []
    for c in range(NC):
        r0, r1 = c * JC, (c + 1) * JC
        wqkv = np.ascontiguousarray(
            np.concatenate([Wq[r0:r1].T, Wk[r0:r1].T, Wv[r0:r1].T], axis=1)
            .astype(ml_dtypes.bfloat16))
        woT = np.ascontiguousarray(
            Wo[:, r0:r1].T.astype(ml_dtypes.bfloat16))
        in_maps.append({"xT": xT, "wqkv": wqkv, "woT": woT, "cs": cs})
    return in_maps


def _gather(results):
    acc = results[0]["out"].astype(np.float64)
    for r in results[1:]:
        acc += r["out"].astype(np.float64)
    out = acc.astype(np.float32).reshape(B, S, DM).transpose(1, 0, 2)
    return np.ascontiguousarray(out)


def kernel(x, rope_emb, Wq, Wk, Wv, Wo, gq, gk):
    in_maps = _prep_inputs(x, rope_emb, Wq, Wk, Wv, Wo, gq, gk)
    nc = _get_program()
    res = bass_utils.run_bass_kernel_spmd(nc, in_maps, core_ids=list(range(NC)))
    return _gather(res.results)


def kernel_profiled(x, rope_emb, Wq, Wk, Wv, Wo, gq, gk):
    """Like kernel() but with NTFF tracing; returns (out, exec_time_ns)."""
    _install_ntff()
    in_maps = _prep_inputs(x, rope_emb, Wq, Wk, Wv, Wo, gq, gk)
    nc = _get_program()
    res = bass_utils.run_bass_kernel_spmd(nc, in_maps, core_ids=list(range(NC)),
                                          trace=True)
    return _gather(res.results), res.exec_time_ns


def _install_ntff():
    import contextlib
    import ctypes
    import types

    if "antenv.axon_hooks" in sys.modules:
        return
    so_path = "/opt/axon/libaxon_pjrt.so"
    try:
        lib = ctypes.CDLL(so_path)
    except OSError:
        return
    if not hasattr(lib, "axon_start_nrt_profile"):
        return
    lib.axon_start_nrt_profile.argtypes = [ctypes.POINTER(ctypes.c_int64),
                                           ctypes.c_size_t]
    lib.axon_start_nrt_profile.restype = ctypes.c_int64
    lib.axon_stop_nrt_profile.argtypes = [ctypes.c_char_p]
    lib.axon_stop_nrt_profile.restype = ctypes.c_int64

    @contextlib.contextmanager
    def hook(output_dir, device_ids):
        import jax
        jax.devices()
        if device_ids:
            ids = (ctypes.c_int64 * len(device_ids))(*device_ids)
            rc = lib.axon_start_nrt_profile(ids, len(device_ids))
        else:
            rc = lib.axon_start_nrt_profile(None, 0)
        if rc != 0:
            raise RuntimeError(f"axon_start_nrt_profile rc={rc}")
        try:
            yield
        finally:
            n = lib.axon_stop_nrt_profile(str(output_dir).encode())
            print(f"ntff profile: {n} file(s) -> {output_dir}", file=sys.stderr)

    mod = types.ModuleType("antenv.axon_hooks")
    _state = {"h": hook}
    mod.get_axon_ntff_profile_hook = lambda: _state["h"]
    mod.set_axon_ntff_profile_hook = lambda h: _state.__setitem__("h", h)
    sys.modules["antenv.axon_hooks"] = mod
